# revision 2
# baseline (speedup 1.0000x reference)
"""Trainium2 Bass kernel for per-sample channel attention (fp8 DoubleRow).

Reference computation (per sample n of 32):
    e  = x[n].reshape(C, HW)                      # C=512, HW=1024
    q  = sigmoid(relu(e @ wq1) @ wq2)             # [C, HW]
    k  = sigmoid(relu(e @ wk1) @ wk2)             # [C, HW]
    v  = sigmoid(relu(e @ wv1) @ wv2)             # [C, HW]
    s  = q @ k.T / sqrt(C)                        # [C, C]
    o  = softmax(s, axis=-1) @ v                  # [C, HW]

Data-parallel over batch N across 8 cores (4 samples each), weights
replicated.  Same transpose-free arrangement as the bf16 version, plus:

fp8 DoubleRow matmuls (perf_mode=DoubleRow, fp8e4 operands): the PE
array virtualizes to 128x256, one DR matmul contracts K=256 (two
128-chunks via 3D APs [128, 2, free]) in ~the cycles of one bf16 N=512
matmul -> heavy matmul count halves vs bf16.

Accuracy (sim absmax-rel 5.2e-3 vs 2e-2 budget; bf16 version was 1e-3):
  - weights pre-scaled x64 on host so fp8 quantization stays out of the
    subnormal range; x pre-scaled x8; relu output scaled x16 into fp8.
    All rescales fold into free ACT scale / DVE tensor_scalar constants.
  - v-path weights split W ~ W_hi + W_lo (both fp8, lo = residual):
    v-weight quantization error is the dominant error term (it biases
    the output directly; q/k weight error only perturbs logits of a
    near-uniform softmax).  The lo terms are extra accumulation matmuls
    into the same PSUM group.
  - sigmoid via tanh: sigmoid(g) = (1+tanh(g/2))/2.  tanh and exp live
    in the same ACT table set ("exp_and_others") so the per-sample
    sigmoid<->exp table reloads (~2.7us each) of the naive form vanish.
      q: ACT tanh -> bf16 tmp, DVE (t+1)*0.5 -> fp8 (q in sigmoid form)
      k,v: raw tanh in fp8.  Scores: s = q @ k.T = 1/2 q @ tk.T +
      (c-only term that cancels in softmax_d).  Output: o = P @ v =
      1/2 + (P @ tv) * (1/(2Z)) since P rows sum to 1.
  - softmax needs no max-subtract: logits s/sqrt(C) have tiny spread
    (~N(0,0.23)), exp() output sits in [0.3, 3] - fp8-friendly.
  - ACT calls operate on [128, 1024] PSUM pairs (two banks) to amortize
    the 352-cycle ACT instruction overhead.
  - output written bf16 (host upcasts), halving output DMA.
"""

import math

import numpy as np
import ml_dtypes

N, C, H, W, R = 32, 512, 32, 32, 4
HW = H * W            # 1024
HID = HW // R         # 256
NCORES = 8
PER = N // NCORES     # samples per core
P = 128               # partitions

KO1 = HW // P         # 8  k-chunks for layer1 / scores contraction
KO2 = HID // P        # 2  k-chunks for layer2
MT_H = HID // P       # 2  m-tiles of h^T
MT_E = HW // P        # 8  m-tiles of q^T/k^T (HW rows)
MT_C = C // P         # 4  m-tiles over C
NH = HW // C          # 2  halves of HW free dim (512 each)

SE = 8.0              # x pre-scale (host)
SW = 64.0             # weight pre-scale (host)
SH = 16.0             # relu-output scale (device, folded into DVE)

_STATE = {}
MM_DT = "fp8"         # kept for test.py compat; ignored


def _build_nc():
    import concourse.bass as bass  # noqa: F401
    import concourse.mybir as mybir
    import concourse.tile as tile
    from concourse import bacc

    f8 = mybir.dt.float8e4
    f32 = mybir.dt.float32
    bf16 = mybir.dt.bfloat16
    DR = mybir.MatmulPerfMode.DoubleRow
    AF = mybir.ActivationFunctionType
    OP = mybir.AluOpType

    nc = bacc.Bacc("TRN2")

    # Inputs pre-swizzled to SBUF-native layout [P, o, m] on host.
    xt = nc.dram_tensor("xt", [PER, P, KO1, C], f8, kind="ExternalInput")
    ws1 = {
        kind: nc.dram_tensor(f"w{kind}1", [P, KO1, HID], f8, kind="ExternalInput")
        for kind in "qkv"
    }
    ws2 = {
        kind: nc.dram_tensor(f"w{kind}2", [P, KO2, HW], f8, kind="ExternalInput")
        for kind in "qkv"
    }
    wv1lo_d = nc.dram_tensor("wv1lo", [P, KO1, HID], f8, kind="ExternalInput")
    wv2lo_d = nc.dram_tensor("wv2lo", [P, KO2, HW], f8, kind="ExternalInput")
    out = nc.dram_tensor("o", [PER, C, HW], bf16, kind="ExternalOutput")

    exp_scale = 0.5 / math.sqrt(C)
    tanh_scale = 0.5 / (SH * SW)
    relu_scale = SH / (SE * SW)

    with tile.TileContext(nc) as tc:
        with (
            tc.tile_pool(name="singles", bufs=1) as singles,
            tc.tile_pool(name="acts", bufs=2) as acts,
            tc.tile_pool(name="hts", bufs=3) as hts,
            tc.tile_pool(name="tmps", bufs=2) as tmps,
            tc.tile_pool(name="obuf", bufs=3) as obuf,
            tc.tile_pool(name="psum", bufs=3, space="PSUM") as psum,
            tc.tile_pool(name="psz", bufs=2, space="PSUM") as psz,
        ):
            w1_sb, w2_sb = {}, {}
            for kind in "qkv":
                w1_sb[kind] = singles.tile(
                    [P, KO1, HID], f8, tag=f"w1{kind}", name=f"w1{kind}"
                )
                w2_sb[kind] = singles.tile(
                    [P, KO2, HW], f8, tag=f"w2{kind}", name=f"w2{kind}"
                )
            wv1lo = singles.tile([P, KO1, HID], f8, tag="wv1lo", name="wv1lo")
            wv2lo = singles.tile([P, KO2, HW], f8, tag="wv2lo", name="wv2lo")

            warm_sb = singles.tile([P, C], f8, tag="warm", name="warm")
            nc.gpsimd.memset(warm_sb, 0.0)
            ones2 = singles.tile([P, 1], f8, tag="ones2", name="ones2")
            nc.vector.memset(ones2, 2.0)

            # First DMA wave in need-order: wq1 + first sample's eT split
            # across sync (HWDGE ~180GB/s) and gpsimd (SWDGE ~80GB/s),
            # then wq2 and the k/v weights behind them.
            nc.sync.dma_start(out=w1_sb["q"], in_=ws1["q"][:])
            eT0 = acts.tile([P, KO1, C], f8, tag="eT", name="eT")
            nc.gpsimd.dma_start(out=eT0[:, 4:, :], in_=xt[0][:, 4:, :])
            nc.sync.dma_start(out=eT0[:, :2, :], in_=xt[0][:, :2, :])
            nc.sync.dma_start(out=eT0[:, 2:4, :], in_=xt[0][:, 2:4, :])
            nc.sync.dma_start(out=w2_sb["q"], in_=ws2["q"][:])
            nc.sync.dma_start(out=w1_sb["k"], in_=ws1["k"][:])
            nc.sync.dma_start(out=w2_sb["k"], in_=ws2["k"][:])
            nc.sync.dma_start(out=w1_sb["v"], in_=ws1["v"][:])
            nc.sync.dma_start(out=wv1lo, in_=wv1lo_d[:])
            nc.sync.dma_start(out=w2_sb["v"], in_=ws2["v"][:])
            nc.sync.dma_start(out=wv2lo, in_=wv2lo_d[:])

            # PE clock warm-up (HAM un-throttles after ~3.4us busy).
            warm_ps = psum.tile([P, 2, C], f32, tag="ps", name="ps")
            for _ in range(10):
                nc.tensor.matmul(
                    warm_ps[:, 0, :], warm_sb[:, :P], warm_sb,
                    start=True, stop=True,
                )

            for s in range(PER):
                if s == 0:
                    eT = eT0
                else:
                    eT = acts.tile([P, KO1, C], f8, tag="eT", name="eT")
                    nc.scalar.dma_start(out=eT, in_=xt[s])

                qT = acts.tile([P, MT_E, C], f8, tag="qT", name="qT")
                kT = acts.tile([P, MT_E, C], f8, tag="kT", name="kT")
                v = acts.tile([P, MT_C, HW], f8, tag="v", name="v")

                for kind in "qkv":
                    split = kind == "v"
                    # layer 1: h^T[r, c] over HW contraction, DR pairs.
                    hT = hts.tile([P, KO2, C], f8, tag="hT", name="hT")
                    ps1 = psum.tile([P, 2, C], f32, tag="ps", name="ps")
                    for m in range(MT_H):
                        l1w = [w1_sb[kind]] + ([wv1lo] if split else [])
                        nmm = 4 * len(l1w)
                        i = 0
                        for wt in l1w:
                            for j in range(4):
                                nc.tensor.matmul(
                                    ps1[:, m, :],
                                    wt[:, 2 * j : 2 * j + 2, m * P : (m + 1) * P],
                                    eT[:, 2 * j : 2 * j + 2, :],
                                    start=(i == 0),
                                    stop=(i == nmm - 1),
                                    perf_mode=DR,
                                )
                                i += 1
                    # relu + x16 rescale, PSUM pair -> fp8 SBUF
                    nc.vector.tensor_scalar(
                        hT[:, :, :], ps1[:, :, :], relu_scale, 0.0,
                        OP.mult, OP.max,
                    )

                    if kind in ("q", "k"):
                        dst = qT if kind == "q" else kT
                        for mp in range(MT_E // 2):
                            ps2 = psum.tile([P, 2, C], f32, tag="ps", name="ps")
                            for h in range(2):
                                m = 2 * mp + h
                                nc.tensor.matmul(
                                    ps2[:, h, :],
                                    w2_sb[kind][:, 0:2, m * P : (m + 1) * P],
                                    hT[:, 0:2, :],
                                    start=True, stop=True,
                                    perf_mode=DR,
                                )
                            if kind == "k":
                                nc.scalar.activation(
                                    dst[:, 2 * mp : 2 * mp + 2, :], ps2,
                                    AF.Tanh, scale=tanh_scale,
                                )
                            else:
                                tq = tmps.tile([P, 2, C], bf16, tag="tq", name="tq")
                                nc.scalar.activation(
                                    tq, ps2, AF.Tanh, scale=tanh_scale
                                )
                                # q = (t+1)/2: back to sigmoid form
                                nc.vector.tensor_scalar(
                                    qT[:, 2 * mp : 2 * mp + 2, :], tq,
                                    1.0, 0.5, OP.add, OP.mult,
                                )
                    else:
                        # v natural: v[d, e] = (h^T).T @ w2 (+ lo residual)
                        for m in range(MT_C):
                            ps2 = psum.tile([P, 2, C], f32, tag="ps", name="ps")
                            for h in range(NH):
                                for i, wt in enumerate([w2_sb["v"], wv2lo]):
                                    nc.tensor.matmul(
                                        ps2[:, h, :],
                                        hT[:, 0:2, m * P : (m + 1) * P],
                                        wt[:, 0:2, h * C : (h + 1) * C],
                                        start=(i == 0),
                                        stop=(i == 1),
                                        perf_mode=DR,
                                    )
                            nc.scalar.activation(
                                v[:, m, :], ps2, AF.Tanh, scale=tanh_scale
                            )

                # scores transposed: s^T[d, c] = (tk^T).T @ q^T (DR pairs),
                # E = exp(s^T * 0.5/sqrt(C)) fused into ACT scale.
                E = acts.tile([P, MT_C, C], f8, tag="E", name="E")
                for mp in range(MT_C // 2):
                    ps = psum.tile([P, 2, C], f32, tag="ps", name="ps")
                    for h in range(2):
                        m = 2 * mp + h
                        for j in range(4):
                            nc.tensor.matmul(
                                ps[:, h, :],
                                kT[:, 2 * j : 2 * j + 2, m * P : (m + 1) * P],
                                qT[:, 2 * j : 2 * j + 2, :],
                                start=(j == 0),
                                stop=(j == 3),
                                perf_mode=DR,
                            )
                    nc.scalar.activation(
                        E[:, 2 * mp : 2 * mp + 2, :], ps, AF.Exp,
                        scale=exp_scale,
                    )

                # softmax denominator: Z2[c] = sum_d 2*E[d, c] (ones=2.0
                # matmuls) -> rz = 1/(2Z) per-partition.
                rz = obuf.tile([P, MT_C], f32, tag="rz", name="rz")
                for m in range(MT_C):
                    pz = psz.tile([P, 1], f32, tag="pz", name="pz")
                    for k in range(MT_C):
                        nc.tensor.matmul(
                            pz,
                            E[:, k, m * P : (m + 1) * P],
                            ones2,
                            start=(k == 0),
                            stop=(k == MT_C - 1),
                        )
                    nc.vector.reciprocal(rz[:, m : m + 1], pz)

                # o[c, e] = (E.T @ tv) * rz[c] + 0.5, bf16 out
                out_r = out[s].rearrange(
                    "(mo p) (h e) -> p mo h e", p=P, h=NH
                )
                for m in range(MT_C):
                    po = psum.tile([P, 2, C], f32, tag="ps", name="ps")
                    for h in range(NH):
                        for j in range(MT_C // 2):
                            nc.tensor.matmul(
                                po[:, h, :],
                                E[:, 2 * j : 2 * j + 2, m * P : (m + 1) * P],
                                v[:, 2 * j : 2 * j + 2, h * C : (h + 1) * C],
                                start=(j == 0),
                                stop=(j == MT_C // 2 - 1),
                                perf_mode=DR,
                            )
                    ob = obuf.tile([P, NH, C], bf16, tag="ob", name="ob")
                    nc.vector.tensor_scalar(
                        ob, po, rz[:, m : m + 1], 0.5, OP.mult, OP.add
                    )
                    oeng = (nc.sync, nc.scalar)[m % 2] if s == PER - 1 else nc.sync
                    oeng.dma_start(out=out_r[:, m, :, :], in_=ob)

    nc.finalize()
    return nc


def _get_nc():
    if "nc" not in _STATE:
        _STATE["nc"] = _build_nc()
    return _STATE["nc"]


def kernel(**inputs):
    f8 = ml_dtypes.float8_e4m3
    x = np.asarray(inputs["x"]).astype(np.float32)

    # host-side reformat to SBUF-native layouts + fp8 quantization:
    #   x:  [N, C, H, W] -> e^T [N, HW, C] * 8 -> [N, P, KO1, C]
    #   w1: [HW, HID] * 64 -> [P, KO1, HID];  w2: [HID, HW] * 64 -> [P, KO2, HW]
    #   v-path weights also ship the fp8 residual (lo) part.
    xt = np.ascontiguousarray(
        (x.reshape(N, C, HW) * np.float32(SE))
        .transpose(0, 2, 1)
        .reshape(N, KO1, P, C)
        .transpose(0, 2, 1, 3)
    ).astype(f8)
    w = {}
    for name, ko, inner in (
        ("wq1", KO1, HID), ("wk1", KO1, HID), ("wv1", KO1, HID),
        ("wq2", KO2, HW), ("wk2", KO2, HW), ("wv2", KO2, HW),
    ):
        a = np.asarray(inputs[name]).astype(np.float32) * np.float32(SW)
        hi = a.astype(f8)
        w[name] = np.ascontiguousarray(
            hi.reshape(ko, P, inner).transpose(1, 0, 2)
        )
        if name in ("wv1", "wv2"):
            lo = (a - hi.astype(np.float32)).astype(f8)
            w[name + "lo"] = np.ascontiguousarray(
                lo.reshape(ko, P, inner).transpose(1, 0, 2)
            )

    nc = _get_nc()

    in_maps = []
    for c in range(NCORES):
        m = {"xt": np.ascontiguousarray(xt[c * PER : (c + 1) * PER])}
        for kind in "qkv":
            m[f"w{kind}1"] = w[f"w{kind}1"]
            m[f"w{kind}2"] = w[f"w{kind}2"]
        m["wv1lo"] = w["wv1lo"]
        m["wv2lo"] = w["wv2lo"]
        in_maps.append(m)

    from concourse.bass_utils import run_bass_kernel_spmd

    res = run_bass_kernel_spmd(
        nc,
        in_maps,
        core_ids=list(range(NCORES)),
        trace=_STATE.get("trace", False),
        **_STATE.get("run_kwargs", {}),
    )
    _STATE["last_result"] = res

    o = np.concatenate([r["o"] for r in res.results], axis=0)
    return o.reshape(N, C, H, W).astype(np.float32)
